# revision 1
# baseline (speedup 1.0000x reference)
"""Mamba block on 8 trn2 NeuronCores — v3.

Sharding: data-parallel over batch (2 groups of 4 cores) x tensor-parallel
over d_inner (4-way, 512 channels/core), [channel, time] layout so the
selective scan runs as `tensor_tensor_scan` along the free (time) axis.

v3 vs baseline:
- Front phases (in_proj, conv+silu, x_proj, AllReduce, dt_proj) run per
  T-half so the two AllReduces overlap the other half's compute.
- Scan-phase muls consolidated: dBx / hc / y+= each run as ONE wide DVE
  op over a [128, 4*2048] concatenated tile per state, with B/C read
  through a stride-0 broadcast view (4 segments share one [128,2048]
  broadcast tile). hc is computed in place on the scan output.
- y for d-tile segment m0 accumulates in PSUM via TensorE identity
  matmuls (start/stop over the 16 states); other segments accumulate on
  DVE, seeded directly at s==0 (no memset, no s==0 add).
- softplus/silu "+1" folded into Ln bias; conv j=0 tap on ScalarE.

Segment order in consolidated tiles: [m1, m2, m3, m0] so the DVE y
accumulation covers a contiguous [128, 3*2048] prefix and the PSUM path
reads the final segment.
"""

import numpy as np

from concourse import bass, mybir, tile
from concourse import bacc
from concourse.bass_utils import run_bass_kernel_spmd

D_MODEL = 1024
D_STATE = 16
D_CONV = 4
D_INNER = 2048
DT_RANK = 64
B_SZ, T_LEN = 2, 2048

TP = 4
DSH = D_INNER // TP         # 512 channels per core
NT = DSH // 128             # 4 d-tiles
H = T_LEN // 2              # 1024
CH = 512

F32 = mybir.dt.float32
F16 = mybir.dt.float16
MUL = mybir.AluOpType.mult
ADD = mybir.AluOpType.add
AF = mybir.ActivationFunctionType

SEG = [3, 0, 1, 2]  # m -> segment index in consolidated tiles ([m1,m2,m3,m0])


def build_graph():
    nc = bacc.Bacc("TRN2", target_bir_lowering=False, num_devices=8)

    hsT = nc.dram_tensor("hsT", [D_MODEL, T_LEN], F16, kind="ExternalInput")
    w_inT = nc.dram_tensor("w_inT", [D_MODEL, 2 * DSH], F16, kind="ExternalInput")
    w_xT = nc.dram_tensor("w_xT", [DSH, DT_RANK + 2 * D_STATE], F16, kind="ExternalInput")
    w_dtT = nc.dram_tensor("w_dtT", [DT_RANK, DSH], F16, kind="ExternalInput")
    w_outT = nc.dram_tensor("w_outT", [DSH, D_MODEL], F16, kind="ExternalInput")
    conv_w = nc.dram_tensor("conv_w", [NT, 128, D_CONV], F32, kind="ExternalInput")
    # vecs columns: 0=conv_b, 1=b_dt, 2=D, 3=-conv_b
    vecs = nc.dram_tensor("vecs", [NT, 128, 4], F32, kind="ExternalInput")
    a_log = nc.dram_tensor("a_log", [NT, 128, D_STATE], F32, kind="ExternalInput")
    onehot = nc.dram_tensor("onehot", [32, 32 * 128], F16, kind="ExternalInput")
    ident = nc.dram_tensor("ident", [128, 128], F16, kind="ExternalInput")
    out_d = nc.dram_tensor("out", [D_MODEL, T_LEN], F32, kind="ExternalOutput")

    def seg_sl(m):
        s0 = SEG[m] * T_LEN
        return slice(s0, s0 + T_LEN)

    with tile.TileContext(nc) as tc:
        with (
            tc.tile_pool(name="wconst", bufs=1) as wconst,
            tc.tile_pool(name="acts", bufs=1) as acts,
            tc.tile_pool(name="outstg", bufs=2) as outstg,
            tc.tile_pool(name="psum_mm", bufs=2, space="PSUM") as psum_mm,
            tc.tile_pool(name="psum_bc", bufs=2, space="PSUM") as psum_bc,
            tc.tile_pool(name="dram", bufs=1, space="DRAM") as dram,
        ):
            # ---- small resident weights/consts ----
            w_x_sb = wconst.tile([128, NT * 96], F16)
            for k in range(NT):
                nc.sync.dma_start(
                    w_x_sb[:, k * 96:(k + 1) * 96],
                    w_xT[k * 128:(k + 1) * 128, :])
            w_dt_sb = wconst.tile([DT_RANK, DSH], F16)
            nc.sync.dma_start(w_dt_sb[:], w_dtT[:])
            w_out_sb = wconst.tile([128, NT * D_MODEL], F16)
            for k in range(NT):
                nc.sync.dma_start(
                    w_out_sb[:, k * D_MODEL:(k + 1) * D_MODEL],
                    w_outT[k * 128:(k + 1) * 128, :])
            conv_w_sb = wconst.tile([128, NT * D_CONV], F32)
            vecs_sb = wconst.tile([128, NT * 4], F32)
            a_log_sb = wconst.tile([128, NT * D_STATE], F32)
            for m in range(NT):
                nc.sync.dma_start(conv_w_sb[:, m * 4:(m + 1) * 4], conv_w[m])
                nc.sync.dma_start(vecs_sb[:, m * 4:(m + 1) * 4], vecs[m])
                nc.sync.dma_start(a_log_sb[:, m * 16:(m + 1) * 16], a_log[m])
            onehot_sb = wconst.tile([32, 32 * 128], F16)
            nc.sync.dma_start(onehot_sb[:], onehot[:])
            ident_sb = wconst.tile([128, 128], F16)
            nc.sync.dma_start(ident_sb[:], ident[:])

            a_sb = wconst.tile([128, NT * D_STATE], F32)
            nc.scalar.activation(a_sb[:], a_log_sb[:], AF.Exp)
            nc.vector.tensor_scalar_mul(a_sb[:], a_sb[:], -1.0)

            # ~5us of dummy matmuls releases the PE HAM clock gate before
            # in_proj arrives (cold PE runs at 1.2 GHz vs 2.4 warm)
            with tc.tile_pool(name="warm", bufs=1, space="PSUM") as warmp:
                wps = warmp.tile([128, CH], F32, name="wps")
                for w in range(12):
                    nc.tensor.matmul(
                        wps[:], onehot_sb[:, 0:128],
                        onehot_sb[:, 0:CH], start=True, stop=True)

            # ---- activations ----
            z_all = [acts.tile([128, T_LEN], F16, name=f"z{m}") for m in range(NT)]
            xc = [acts.tile([128, T_LEN], F16, name=f"xc{m}") for m in range(NT)]
            dt_sb = [acts.tile([128, T_LEN], F16, name=f"dt{m}") for m in range(NT)]
            u_all = acts.tile([128, NT * T_LEN], F16)
            y_all = acts.tile([128, 3 * T_LEN], F16)   # segments m1,m2,m3
            y0 = acts.tile([128, T_LEN], F16)          # m0 (PSUM path)
            bc_f16 = acts.tile([32, T_LEN], F16)

            cc_in = [dram.tile([96, H], F16, name=f"ccin{h}") for h in range(2)]
            cc_out = [dram.tile([96, H], F16, name=f"ccout{h}") for h in range(2)]

            with (
                tc.tile_pool(name="xpool", bufs=1) as xpool,
                tc.tile_pool(name="winp", bufs=1) as winp,
                tc.tile_pool(name="hspool", bufs=2) as hspool,
                tc.tile_pool(name="convp", bufs=2) as convp,
                tc.tile_pool(name="xdblp", bufs=1) as xdblp,
                tc.tile_pool(name="sppool", bufs=2) as sppool,
            ):
                x_all = [xpool.tile([128, T_LEN + 3], F16, name=f"x{m}")
                         for m in range(NT)]
                w_in_sb = winp.tile([128, 8 * 1024], F16)
                for k in range(8):
                    nc.sync.dma_start(
                        w_in_sb[:, k * 1024:(k + 1) * 1024],
                        w_inT[k * 128:(k + 1) * 128, :])
                for m in range(NT):
                    nc.any.memset(x_all[m][:, 0:3], 0.0)

                for h in range(2):
                    hc0 = h * H
                    # ---- P1: in_proj ----
                    for n in range(2):
                        cols = slice(hc0 + n * CH, hc0 + (n + 1) * CH)
                        hs_n = hspool.tile([128, 8 * CH], F16, tag="hs")
                        for k in range(8):
                            nc.sync.dma_start(
                                hs_n[:, k * CH:(k + 1) * CH],
                                hsT[k * 128:(k + 1) * 128, cols])
                        for mo in range(8):
                            ps = psum_mm.tile([128, CH], F32, tag="ps")
                            for k in range(8):
                                nc.tensor.matmul(
                                    ps[:],
                                    w_in_sb[:, k * 1024 + mo * 128:
                                            k * 1024 + (mo + 1) * 128],
                                    hs_n[:, k * CH:(k + 1) * CH],
                                    start=(k == 0), stop=(k == 7))
                            if mo < NT:
                                nc.vector.tensor_copy(
                                    x_all[mo][:, 3 + hc0 + n * CH:
                                              3 + hc0 + (n + 1) * CH], ps[:])
                            else:
                                nc.vector.tensor_copy(
                                    z_all[mo - NT][:, cols], ps[:])

                    # ---- P2: conv + silu(x); gate = silu(z) ----
                    for m in range(NT):
                        craw = convp.tile([128, H], F16, tag="craw")
                        nc.scalar.activation(
                            craw[:], x_all[m][:, hc0:hc0 + H], AF.Copy,
                            scale=conv_w_sb[:, m * 4: m * 4 + 1])
                        for j in range(1, D_CONV):
                            nc.vector.scalar_tensor_tensor(
                                craw[:], x_all[m][:, hc0 + j:hc0 + j + H],
                                conv_w_sb[:, m * 4 + j: m * 4 + j + 1], craw[:],
                                op0=MUL, op1=ADD)
                        t1 = convp.tile([128, H], F16, tag="t1")
                        nc.scalar.activation(
                            t1[:], craw[:], AF.Exp, scale=-1.0,
                            bias=vecs_sb[:, m * 4 + 3: m * 4 + 4])
                        t2 = convp.tile([128, H], F16, tag="t2")
                        nc.scalar.activation(t2[:], t1[:], AF.Ln, bias=1.0)
                        sg = convp.tile([128, H], F16, tag="sg")
                        nc.scalar.activation(sg[:], t2[:], AF.Exp, scale=-1.0)
                        nc.vector.scalar_tensor_tensor(
                            xc[m][:, hc0:hc0 + H], craw[:],
                            vecs_sb[:, m * 4: m * 4 + 1], sg[:],
                            op0=ADD, op1=MUL)

                    # ---- P3: x_proj partial + AllReduce(h) (f16) ----
                    xdbl_h = xdblp.tile([96, H], F16, tag="xd")
                    for n in range(2):
                        cols = slice(hc0 + n * CH, hc0 + (n + 1) * CH)
                        psx = psum_mm.tile([128, CH], F32, tag="ps")
                        for k in range(NT):
                            nc.tensor.matmul(
                                psx[0:96, :], w_x_sb[:, k * 96:(k + 1) * 96],
                                xc[k][:, cols],
                                start=(k == 0), stop=(k == NT - 1))
                        nc.vector.tensor_copy(
                            xdbl_h[:, n * CH:(n + 1) * CH], psx[0:96, :])
                    nc.sync.dma_start(cc_in[h][:], xdbl_h[:])
                    nc.gpsimd.collective_compute(
                        "AllReduce", ADD,
                        replica_groups=[[0, 1, 2, 3], [4, 5, 6, 7]],
                        ins=[cc_in[h].opt()], outs=[cc_out[h].opt()])

                # ---- P4 per half: AR readback, dt_proj+softplus, u ----
                for h in range(2):
                    hc0 = h * H
                    xdbl16 = xdblp.tile([DT_RANK, H], F16, tag="x16")
                    nc.sync.dma_start(xdbl16[:], cc_out[h][0:64, :])
                    nc.sync.dma_start(bc_f16[:, hc0:hc0 + H], cc_out[h][64:96, :])
                    for m in range(NT):
                        for n in range(2):
                            cols = slice(hc0 + n * CH, hc0 + (n + 1) * CH)
                            psd = psum_mm.tile([128, CH], F32, tag="ps")
                            nc.tensor.matmul(
                                psd[:], w_dt_sb[:, m * 128:(m + 1) * 128],
                                xdbl16[:, n * CH:(n + 1) * CH],
                                start=True, stop=True)
                            sp = sppool.tile([128, CH], F32, tag="sp")
                            nc.scalar.activation(
                                sp[:], psd[:], AF.Exp,
                                bias=vecs_sb[:, m * 4 + 1: m * 4 + 2])
                            nc.scalar.activation(
                                dt_sb[m][:, cols], sp[:], AF.Ln, bias=1.0)
                        nc.vector.tensor_tensor(
                            u_all[:, SEG[m] * T_LEN + hc0:
                                  SEG[m] * T_LEN + hc0 + H],
                            dt_sb[m][:, hc0:hc0 + H],
                            xc[m][:, hc0:hc0 + H], op=MUL)


            # ---- P6: selective scan over states (full T) ----
            with (
                tc.tile_pool(name="bcast", bufs=2) as bcast,
                tc.tile_pool(name="dap", bufs=2) as dap,
                tc.tile_pool(name="dbxp", bufs=1) as dbxp,
                tc.tile_pool(name="hp", bufs=2) as hp,
                tc.tile_pool(name="psum_y", bufs=1, space="PSUM") as psum_y,
            ):
                y0_ps = psum_y.tile([128, T_LEN], F32, name="y0_ps")
                s1_scan = None

                # Early block: state 0's first column-half only needs
                # AR(h0)-derived data (bc rows, dt, u for cols 0:H), which is
                # ready while AR(h1) is still in flight — fill that DVE idle
                # window. The h1 half chains via the scan carry in the main
                # loop below.
                e_Bb = bcast.tile([128, T_LEN], F16, tag="Bb")
                e_Cb = bcast.tile([128, T_LEN], F16, tag="Cb")
                for n in range(2):
                    cols = slice(n * CH, (n + 1) * CH)
                    psb = psum_bc.tile([128, CH], F32, tag="psb")
                    nc.tensor.matmul(
                        psb[:], onehot_sb[:, 0:128],
                        bc_f16[:, cols], start=True, stop=True)
                    nc.any.tensor_copy(e_Bb[:, cols], psb[:])
                    psc = psum_bc.tile([128, CH], F32, tag="psb")
                    nc.tensor.matmul(
                        psc[:], onehot_sb[:, 16 * 128:17 * 128],
                        bc_f16[:, cols], start=True, stop=True)
                    nc.any.tensor_copy(e_Cb[:, cols], psc[:])
                e_dA = dap.tile([128, NT * T_LEN], F16, tag="dA")
                e_dBx = dbxp.tile([128, NT * T_LEN], F16, tag="dBx")
                e_hh = hp.tile([128, NT * T_LEN], F16, tag="h")
                for m in range(NT):
                    s0 = SEG[m] * T_LEN
                    nc.scalar.activation(
                        e_dA[:, s0:s0 + H], dt_sb[m][:, 0:H], AF.Exp,
                        scale=a_sb[:, m * 16: m * 16 + 1])
                    nc.vector.tensor_tensor(
                        e_dBx[:, s0:s0 + H], u_all[:, s0:s0 + H],
                        e_Bb[:, 0:H], op=MUL)
                    nc.vector.tensor_tensor_scan(
                        e_hh[:, s0:s0 + H], e_dA[:, s0:s0 + H],
                        e_dBx[:, s0:s0 + H], initial=0.0, op0=MUL, op1=ADD)

                for s in range(D_STATE):
                    if s == 2:
                        # gate = silu(z), deferred into the scan window where
                        # the scalar engine has slack; pinned behind the s==1
                        # scans so the scheduler cannot pull it forward into
                        # the AR(h1)->softplus critical chain
                        for m in range(NT):
                            g1 = bcast.tile([128, T_LEN], F16, tag="Bb")
                            act1 = nc.scalar.activation(
                                g1[:], z_all[m][:], AF.Exp, scale=-1.0)
                            tile.add_dep_helper(
                                act1.ins, s1_scan.ins, sync=False,
                                reason="defer gate silu past scan start")
                            g2 = bcast.tile([128, T_LEN], F16, tag="Cb")
                            nc.scalar.activation(g2[:], g1[:], AF.Ln, bias=1.0)
                            nc.scalar.activation(g1[:], g2[:], AF.Exp, scale=-1.0)
                            nc.vector.tensor_tensor(
                                z_all[m][:], z_all[m][:], g1[:], op=MUL)
                    if s == 0:
                        # finish state 0: h1 half only, chained via scan carry
                        Bb, Cb, dA, dBx, hh = e_Bb, e_Cb, e_dA, e_dBx, e_hh
                        for n in range(2, 4):
                            cols = slice(n * CH, (n + 1) * CH)
                            psb = psum_bc.tile([128, CH], F32, tag="psb")
                            nc.tensor.matmul(
                                psb[:], onehot_sb[:, 0:128],
                                bc_f16[:, cols], start=True, stop=True)
                            nc.any.tensor_copy(Bb[:, cols], psb[:])
                            psc = psum_bc.tile([128, CH], F32, tag="psb")
                            nc.tensor.matmul(
                                psc[:], onehot_sb[:, 16 * 128:17 * 128],
                                bc_f16[:, cols], start=True, stop=True)
                            nc.any.tensor_copy(Cb[:, cols], psc[:])
                        for m in range(NT):
                            c0 = SEG[m] * T_LEN
                            nc.scalar.activation(
                                dA[:, c0 + H:c0 + T_LEN], dt_sb[m][:, H:],
                                AF.Exp, scale=a_sb[:, m * 16: m * 16 + 1])
                            nc.vector.tensor_tensor(
                                dBx[:, c0 + H:c0 + T_LEN],
                                u_all[:, c0 + H:c0 + T_LEN],
                                Bb[:, H:], op=MUL)
                            nc.vector.tensor_tensor_scan(
                                hh[:, c0 + H:c0 + T_LEN],
                                dA[:, c0 + H:c0 + T_LEN],
                                dBx[:, c0 + H:c0 + T_LEN],
                                initial=hh[:, c0 + H - 1:c0 + H],
                                op0=MUL, op1=ADD)
                        Cb4 = Cb[:].unsqueeze(1).to_broadcast([128, NT, T_LEN])
                    else:
                        Bb = bcast.tile([128, T_LEN], F16, tag="Bb")
                        Cb = bcast.tile([128, T_LEN], F16, tag="Cb")
                        for n in range(4):
                            cols = slice(n * CH, (n + 1) * CH)
                            psb = psum_bc.tile([128, CH], F32, tag="psb")
                            nc.tensor.matmul(
                                psb[:], onehot_sb[:, s * 128:(s + 1) * 128],
                                bc_f16[:, cols], start=True, stop=True)
                            nc.any.tensor_copy(Bb[:, cols], psb[:])
                            psc = psum_bc.tile([128, CH], F32, tag="psb")
                            nc.tensor.matmul(
                                psc[:], onehot_sb[:, (16 + s) * 128:(17 + s) * 128],
                                bc_f16[:, cols], start=True, stop=True)
                            nc.any.tensor_copy(Cb[:, cols], psc[:])
                        Bb4 = Bb[:].unsqueeze(1).to_broadcast([128, NT, T_LEN])
                        Cb4 = Cb[:].unsqueeze(1).to_broadcast([128, NT, T_LEN])

                        dA = dap.tile([128, NT * T_LEN], F16, tag="dA")
                        for m in range(NT):
                            nc.scalar.activation(
                                dA[:, seg_sl(m)], dt_sb[m][:], AF.Exp,
                                scale=a_sb[:, m * 16 + s: m * 16 + s + 1])
                        dBx = dbxp.tile([128, NT * T_LEN], F16, tag="dBx")
                        nc.vector.tensor_tensor(
                            dBx[:].rearrange("p (a b) -> p a b", a=NT),
                            u_all[:].rearrange("p (a b) -> p a b", a=NT),
                            Bb4, op=MUL)
                        hh = hp.tile([128, NT * T_LEN], F16, tag="h")
                        for m in range(NT):
                            si = nc.vector.tensor_tensor_scan(
                                hh[:, seg_sl(m)], dA[:, seg_sl(m)],
                                dBx[:, seg_sl(m)], initial=0.0, op0=MUL, op1=ADD)
                            if s == 1 and m == 0:
                                s1_scan = si
                    last = s == D_STATE - 1
                    if s == 0:
                        # seed y directly: y = h*C (no memset / add)
                        nc.vector.tensor_tensor(
                            y_all[:].rearrange("p (a b) -> p a b", a=3),
                            hh[:, 0:3 * T_LEN].rearrange("p (a b) -> p a b", a=3),
                            Cb[:].unsqueeze(1).to_broadcast([128, 3, T_LEN]),
                            op=MUL)
                        nc.vector.tensor_tensor(
                            hh[:, 3 * T_LEN:], hh[:, 3 * T_LEN:], Cb[:], op=MUL)
                        for n in range(4):
                            nc.tensor.matmul(
                                y0_ps[:, n * CH:(n + 1) * CH], ident_sb[:],
                                hh[:, 3 * T_LEN + n * CH: 3 * T_LEN + (n + 1) * CH],
                                start=True, stop=False)
                    elif not last:
                        # hc in place on h, then one wide accumulate
                        nc.vector.tensor_tensor(
                            hh[:].rearrange("p (a b) -> p a b", a=NT),
                            hh[:].rearrange("p (a b) -> p a b", a=NT),
                            Cb4, op=MUL)
                        nc.vector.tensor_tensor(
                            y_all[:], y_all[:], hh[:, 0:3 * T_LEN], op=ADD)
                        for n in range(4):
                            nc.tensor.matmul(
                                y0_ps[:, n * CH:(n + 1) * CH], ident_sb[:],
                                hh[:, 3 * T_LEN + n * CH: 3 * T_LEN + (n + 1) * CH],
                                start=False, stop=False)
                    else:
                        # last state: finish per column-quarter and flow each
                        # quarter straight into gate + out_proj so the tail
                        # overlaps the remaining scan work
                        for n in range(4):
                            c0 = n * CH
                            cols = slice(c0, c0 + CH)
                            for m in range(NT):
                                sl = slice(SEG[m] * T_LEN + c0,
                                           SEG[m] * T_LEN + c0 + CH)
                                nc.vector.tensor_tensor(
                                    hh[:, sl], hh[:, sl],
                                    Cb[:, cols], op=MUL)
                                if m != 0:
                                    nc.vector.tensor_tensor(
                                        y_all[:, sl], y_all[:, sl],
                                        hh[:, sl], op=ADD)
                            nc.tensor.matmul(
                                y0_ps[:, cols], ident_sb[:],
                                hh[:, 3 * T_LEN + c0: 3 * T_LEN + c0 + CH],
                                start=False, stop=True)
                            nc.any.tensor_copy(y0[:, cols], y0_ps[:, cols])
                            # gate for this quarter
                            for m in range(NT):
                                ysl = (y0[:, cols] if m == 0
                                       else y_all[:, SEG[m] * T_LEN + c0:
                                                  SEG[m] * T_LEN + c0 + CH])
                                nc.vector.scalar_tensor_tensor(
                                    xc[m][:, cols], xc[m][:, cols],
                                    vecs_sb[:, m * 4 + 2: m * 4 + 3],
                                    ysl, op0=MUL, op1=ADD)
                                nc.vector.tensor_tensor(
                                    xc[m][:, cols], xc[m][:, cols],
                                    z_all[m][:, cols], op=MUL)
                            # out_proj for this quarter
                            for mo in range(8):
                                pso = psum_mm.tile([128, CH], F32, tag="ps")
                                for k in range(NT):
                                    nc.tensor.matmul(
                                        pso[:],
                                        w_out_sb[:, k * D_MODEL + mo * 128:
                                                 k * D_MODEL + (mo + 1) * 128],
                                        xc[k][:, cols],
                                        start=(k == 0), stop=(k == NT - 1))
                                ot = outstg.tile([128, CH], F32, tag="ot")
                                nc.any.tensor_copy(ot[:], pso[:])
                                nc.sync.dma_start(
                                    out_d[mo * 128:(mo + 1) * 128, cols],
                                    ot[:])

    nc.finalize()
    return nc


def _onehot():
    oh = np.zeros((32, 32 * 128), np.float16)
    for s in range(32):
        oh[s, s * 128:(s + 1) * 128] = 1.0
    return oh


def make_in_maps(hidden_states, W_in, conv_w, conv_b, W_x, W_dt, b_dt, A_log, D, W_out):
    f16 = np.float16
    in_maps = []
    for core in range(8):
        g, r = divmod(core, TP)
        sh = slice(r * DSH, (r + 1) * DSH)
        m = {
            "hsT": np.ascontiguousarray(hidden_states[g].T).astype(f16),
            "w_inT": np.ascontiguousarray(
                np.concatenate([W_in[sh], W_in[D_INNER + r * DSH: D_INNER + (r + 1) * DSH]], 0).T).astype(f16),
            "w_xT": np.ascontiguousarray(W_x[:, sh].T).astype(f16),
            "w_dtT": np.ascontiguousarray(W_dt[sh].T).astype(f16),
            "w_outT": np.ascontiguousarray(W_out[:, sh].T).astype(f16),
            "conv_w": np.ascontiguousarray(conv_w[sh, 0, :]).reshape(NT, 128, D_CONV).astype(np.float32),
            "vecs": np.stack([conv_b[sh], b_dt[sh], D[sh], -conv_b[sh]], -1).reshape(NT, 128, 4).astype(np.float32),
            "a_log": np.ascontiguousarray(A_log[sh]).reshape(NT, 128, D_STATE).astype(np.float32),
            "onehot": _onehot(),
            "ident": np.eye(128, dtype=f16),
        }
        in_maps.append(m)
    return in_maps


_NC_CACHE = {}


def kernel(**inputs):
    inputs = {k: np.asarray(v) for k, v in inputs.items()}
    if "nc" not in _NC_CACHE:
        _NC_CACHE["nc"] = build_graph()
    nc = _NC_CACHE["nc"]
    in_maps = make_in_maps(**inputs)
    res = run_bass_kernel_spmd(nc, in_maps, core_ids=list(range(8)))
    outs = res.results
    full = np.zeros((B_SZ, T_LEN, D_MODEL), np.float32)
    for g in range(B_SZ):
        acc = np.zeros((D_MODEL, T_LEN), np.float32)
        for r in range(TP):
            acc += np.asarray(outs[g * TP + r]["out"], np.float32)
        full[g] = acc.T
    return full



# revision 5
# speedup vs baseline: 1.0899x; 1.0899x over previous
"""Mamba block on 8 trn2 NeuronCores — v4.

Sharding: data-parallel over batch (2 groups of 4 cores) x tensor-parallel
over d_inner (4-way, 512 channels/core), [channel, time] layout so the
selective scan runs as `tensor_tensor_scan` along the free (time) axis.

v4 vs v3 (all driven by the DVE being the bottleneck at 77% occupancy):
- B/C per-state broadcasts now run on the DMA engines (stride-0
  partition-broadcast reads straight from the AllReduce output in DRAM)
  instead of PE one-hot matmuls + ScalarE PSUM->SBUF copies. This frees
  2 PSUM banks and ~5us/state of ScalarE.
- Depthwise conv runs on the PE as 4 accumulating diag-matrix matmuls
  (diag built once from `ident` scaled per-partition by the conv tap),
  replacing 24 DVE scalar_tensor_tensor ops. SiLU is applied directly
  PSUM->SBUF on ScalarE via the dedicated Silu activation table entry
  (replaces the 3-op Exp/Ln/Exp chains).
- The gate silu(z) is applied by ScalarE directly on the in_proj PSUM
  output (zg = Silu(z)), removing the DVE z-cast, the deferred 3-op
  sigmoid chain, and the z*sg DVE multiply.
- y accumulation: segment m0 (full T) and the h0 half of segment m1
  accumulate in PSUM via TensorE identity matmuls using the banks freed
  by the DMA broadcasts; the DVE y+= covers the remaining contiguous
  [128, 5*1024] range.
- D skip-connection folded into xc in place with a 4x-mode
  tensor_scalar_mul after u is computed, so the tail per quarter is two
  2x-mode tensor_tensor ops (add y, mul zg).
"""

import numpy as np

from concourse import bass, mybir, tile
from concourse import bacc
from concourse.bass_utils import run_bass_kernel_spmd

D_MODEL = 1024
D_STATE = 16
D_CONV = 4
D_INNER = 2048
DT_RANK = 64
B_SZ, T_LEN = 2, 2048

TP = 4
DSH = D_INNER // TP         # 512 channels per core
NT = DSH // 128             # 4 d-tiles
H = T_LEN // 2              # 1024
CH = 512

F32 = mybir.dt.float32
F16 = mybir.dt.float16
MUL = mybir.AluOpType.mult
ADD = mybir.AluOpType.add
AF = mybir.ActivationFunctionType

SEG = [3, 0, 1, 2]  # m -> segment index in consolidated tiles ([m1,m2,m3,m0])


def build_graph():
    nc = bacc.Bacc("TRN2", target_bir_lowering=False, num_devices=8)

    hsT = nc.dram_tensor("hsT", [D_MODEL, T_LEN], F16, kind="ExternalInput")
    w_inT = nc.dram_tensor("w_inT", [D_MODEL, 2 * DSH], F16, kind="ExternalInput")
    w_xT = nc.dram_tensor("w_xT", [DSH, DT_RANK + 2 * D_STATE], F16, kind="ExternalInput")
    w_dtT = nc.dram_tensor("w_dtT", [DT_RANK, DSH], F16, kind="ExternalInput")
    w_outT = nc.dram_tensor("w_outT", [DSH, D_MODEL], F16, kind="ExternalInput")
    conv_w = nc.dram_tensor("conv_w", [NT, 128, D_CONV], F32, kind="ExternalInput")
    # vecs columns: 0=conv_b, 1=b_dt, 2=D, 3=unused
    vecs = nc.dram_tensor("vecs", [NT, 128, 4], F32, kind="ExternalInput")
    a_log = nc.dram_tensor("a_log", [NT, 128, D_STATE], F32, kind="ExternalInput")
    ident = nc.dram_tensor("ident", [128, 128], F16, kind="ExternalInput")
    out_d = nc.dram_tensor("out", [D_MODEL, T_LEN], F32, kind="ExternalOutput")

    def seg_sl(m):
        s0 = SEG[m] * T_LEN
        return slice(s0, s0 + T_LEN)

    with tile.TileContext(nc) as tc:
        with (
            tc.tile_pool(name="wconst", bufs=1) as wconst,
            tc.tile_pool(name="acts", bufs=1) as acts,
            tc.tile_pool(name="outstg", bufs=2) as outstg,
            tc.tile_pool(name="psum_mm", bufs=2, space="PSUM") as psum_mm,
            tc.tile_pool(name="dram", bufs=1, space="DRAM") as dram,
        ):
            # ---- small resident weights/consts ----
            w_x_sb = wconst.tile([128, NT * 96], F16)
            for k in range(NT):
                nc.sync.dma_start(
                    w_x_sb[:, k * 96:(k + 1) * 96],
                    w_xT[k * 128:(k + 1) * 128, :])
            w_dt_sb = wconst.tile([DT_RANK, DSH], F16)
            nc.sync.dma_start(w_dt_sb[:], w_dtT[:])
            w_out_sb = wconst.tile([128, NT * D_MODEL], F16)
            for k in range(NT):
                nc.sync.dma_start(
                    w_out_sb[:, k * D_MODEL:(k + 1) * D_MODEL],
                    w_outT[k * 128:(k + 1) * 128, :])
            conv_w_sb = wconst.tile([128, NT * 4], F32)
            vecs_sb = wconst.tile([128, NT * 4], F32)
            a_log_sb = wconst.tile([128, NT * D_STATE], F32)
            for m in range(NT):
                nc.sync.dma_start(conv_w_sb[:, m * 4:(m + 1) * 4], conv_w[m])
                nc.sync.dma_start(vecs_sb[:, m * 4:(m + 1) * 4], vecs[m])
                nc.sync.dma_start(a_log_sb[:, m * 16:(m + 1) * 16], a_log[m])
            ident_sb = wconst.tile([128, 128], F16)
            nc.sync.dma_start(ident_sb[:], ident[:])

            a_sb = wconst.tile([128, NT * D_STATE], F32)
            nc.scalar.activation(a_sb[:], a_log_sb[:], AF.Exp)
            nc.vector.tensor_scalar_mul(a_sb[:], a_sb[:], -1.0)

            # conv tap diag matrices: diag(conv_w[:, j]) per d-tile
            diags = wconst.tile([128, NT * 4 * 128], F16)
            for m in range(NT):
                for j in range(4):
                    nc.scalar.activation(
                        diags[:, (m * 4 + j) * 128:(m * 4 + j + 1) * 128],
                        ident_sb[:], AF.Copy,
                        scale=conv_w_sb[:, m * 4 + j: m * 4 + j + 1])

            # ~5us of dummy matmuls releases the PE HAM clock gate before
            # in_proj arrives (cold PE runs at 1.2 GHz vs 2.4 warm)
            with tc.tile_pool(name="warm", bufs=1, space="PSUM") as warmp:
                wps = warmp.tile([128, CH], F32, name="wps")
                for w in range(12):
                    nc.tensor.matmul(
                        wps[:], ident_sb[:], w_out_sb[:, 0:CH],
                        start=True, stop=True)

            # ---- activations ----
            zg_all = [acts.tile([128, T_LEN], F16, name=f"zg{m}") for m in range(NT)]
            xc = [acts.tile([128, T_LEN], F16, name=f"xc{m}") for m in range(NT)]
            dt_sb = [acts.tile([128, T_LEN], F16, name=f"dt{m}") for m in range(NT)]
            u_all = acts.tile([128, NT * T_LEN], F16)
            # y for segments m1(2nd half),m2,m3: global cols 1024..6144
            y_all = acts.tile([128, 5 * H], F16)
            y0 = acts.tile([128, T_LEN], F16)          # m0 (PSUM path)
            y1a = acts.tile([128, H], F16)             # m1 h0-half (PSUM path)

            cc_in = [dram.tile([96, H], F16, name=f"ccin{h}") for h in range(2)]
            cc_out = [dram.tile([96, H], F16, name=f"ccout{h}") for h in range(2)]

            with (
                tc.tile_pool(name="xpool", bufs=1) as xpool,
                tc.tile_pool(name="winp", bufs=1) as winp,
                tc.tile_pool(name="hspool", bufs=2) as hspool,
                tc.tile_pool(name="psum_cv", bufs=2, space="PSUM") as psum_cv,
                tc.tile_pool(name="xdblp", bufs=1) as xdblp,
                tc.tile_pool(name="sppool", bufs=2) as sppool,
            ):
                x_all = [xpool.tile([128, T_LEN + 3], F16, name=f"x{m}")
                         for m in range(NT)]
                w_in_sb = winp.tile([128, 8 * 1024], F16)
                for k in range(8):
                    nc.sync.dma_start(
                        w_in_sb[:, k * 1024:(k + 1) * 1024],
                        w_inT[k * 128:(k + 1) * 128, :])
                for m in range(NT):
                    nc.any.memset(x_all[m][:, 0:3], 0.0)

                for h in range(2):
                    hc0 = h * H
                    # ---- P1: in_proj ----
                    for n in range(2):
                        cols = slice(hc0 + n * CH, hc0 + (n + 1) * CH)
                        hs_n = hspool.tile([128, 8 * CH], F16, tag="hs")
                        for k in range(8):
                            nc.sync.dma_start(
                                hs_n[:, k * CH:(k + 1) * CH],
                                hsT[k * 128:(k + 1) * 128, cols])
                        for mo in range(8):
                            ps = psum_mm.tile([128, CH], F32, tag="ps")
                            for k in range(8):
                                nc.tensor.matmul(
                                    ps[:],
                                    w_in_sb[:, k * 1024 + mo * 128:
                                            k * 1024 + (mo + 1) * 128],
                                    hs_n[:, k * CH:(k + 1) * CH],
                                    start=(k == 0), stop=(k == 7))
                            if mo < NT:
                                nc.vector.tensor_copy(
                                    x_all[mo][:, 3 + hc0 + n * CH:
                                              3 + hc0 + (n + 1) * CH], ps[:])
                            else:
                                # gate: zg = silu(z) straight from PSUM
                                nc.scalar.activation(
                                    zg_all[mo - NT][:, cols], ps[:], AF.Silu)

                    # ---- P2: conv on PE + silu(x) from PSUM ----
                    for m in range(NT):
                        cps = psum_cv.tile([128, H], F32, tag="cv")
                        for j in range(D_CONV):
                            for c in range(2):
                                nc.tensor.matmul(
                                    cps[:, c * CH:(c + 1) * CH],
                                    diags[:, (m * 4 + j) * 128:
                                          (m * 4 + j + 1) * 128],
                                    x_all[m][:, hc0 + j + c * CH:
                                             hc0 + j + c * CH + CH],
                                    start=(j == 0), stop=(j == D_CONV - 1))
                        nc.scalar.activation(
                            xc[m][:, hc0:hc0 + H], cps[:], AF.Silu,
                            bias=vecs_sb[:, m * 4: m * 4 + 1])

                    # ---- P3: x_proj partial + AllReduce(h) (f16) ----
                    xdbl_h = xdblp.tile([96, H], F16, tag="xd")
                    for n in range(2):
                        cols = slice(hc0 + n * CH, hc0 + (n + 1) * CH)
                        psx = psum_mm.tile([128, CH], F32, tag="ps")
                        for k in range(NT):
                            nc.tensor.matmul(
                                psx[0:96, :], w_x_sb[:, k * 96:(k + 1) * 96],
                                xc[k][:, cols],
                                start=(k == 0), stop=(k == NT - 1))
                        nc.vector.tensor_copy(
                            xdbl_h[:, n * CH:(n + 1) * CH], psx[0:96, :])
                    nc.sync.dma_start(cc_in[h][:], xdbl_h[:])
                    nc.gpsimd.collective_compute(
                        "AllReduce", ADD,
                        replica_groups=[[0, 1, 2, 3], [4, 5, 6, 7]],
                        ins=[cc_in[h].opt()], outs=[cc_out[h].opt()])

                # ---- P4 per half: AR readback, dt_proj+softplus, u, D-fold ----
                for h in range(2):
                    hc0 = h * H
                    xdbl16 = xdblp.tile([DT_RANK, H], F16, tag="x16")
                    nc.sync.dma_start(xdbl16[:], cc_out[h][0:64, :])
                    for m in range(NT):
                        for n in range(2):
                            cols = slice(hc0 + n * CH, hc0 + (n + 1) * CH)
                            psd = psum_mm.tile([128, CH], F32, tag="ps")
                            nc.tensor.matmul(
                                psd[:], w_dt_sb[:, m * 128:(m + 1) * 128],
                                xdbl16[:, n * CH:(n + 1) * CH],
                                start=True, stop=True)
                            sp = sppool.tile([128, CH], F32, tag="sp")
                            nc.scalar.activation(
                                sp[:], psd[:], AF.Exp,
                                bias=vecs_sb[:, m * 4 + 1: m * 4 + 2])
                            nc.scalar.activation(
                                dt_sb[m][:, cols], sp[:], AF.Ln, bias=1.0)
                        nc.vector.tensor_tensor(
                            u_all[:, SEG[m] * T_LEN + hc0:
                                  SEG[m] * T_LEN + hc0 + H],
                            dt_sb[m][:, hc0:hc0 + H],
                            xc[m][:, hc0:hc0 + H], op=MUL)
                        # fold D into xc in place (xc no longer needed raw)
                        nc.vector.tensor_scalar_mul(
                            xc[m][:, hc0:hc0 + H], xc[m][:, hc0:hc0 + H],
                            vecs_sb[:, m * 4 + 2: m * 4 + 3])

            # ---- P6: selective scan over states (full T) ----
            with (
                tc.tile_pool(name="bcast", bufs=2) as bcast,
                tc.tile_pool(name="dap", bufs=2) as dap,
                tc.tile_pool(name="dbxp", bufs=1) as dbxp,
                tc.tile_pool(name="hp", bufs=2) as hp,
                tc.tile_pool(name="psum_y", bufs=1, space="PSUM") as psum_y,
            ):
                y0_ps = psum_y.tile([128, T_LEN], F32, name="y0_ps")
                y1a_ps = psum_y.tile([128, H], F32, name="y1a_ps")

                # Early block: state 0's h0 half only needs AR(h0)-derived
                # data, ready while AR(h1) is still in flight.
                e_Bb = bcast.tile([128, T_LEN], F16, tag="Bb")
                e_Cb = bcast.tile([128, T_LEN], F16, tag="Cb")
                nc.sync.dma_start(
                    e_Bb[:, 0:H], cc_out[0][64:65, :].to_broadcast([128, H]))
                nc.sync.dma_start(
                    e_Cb[:, 0:H], cc_out[0][80:81, :].to_broadcast([128, H]))
                e_dA = dap.tile([128, NT * T_LEN], F16, tag="dA")
                e_dBx = dbxp.tile([128, NT * T_LEN], F16, tag="dBx")
                e_hh = hp.tile([128, NT * T_LEN], F16, tag="h")
                for m in range(NT):
                    s0 = SEG[m] * T_LEN
                    nc.scalar.activation(
                        e_dA[:, s0:s0 + H], dt_sb[m][:, 0:H], AF.Exp,
                        scale=a_sb[:, m * 16: m * 16 + 1])
                    nc.vector.tensor_tensor(
                        e_dBx[:, s0:s0 + H], u_all[:, s0:s0 + H],
                        e_Bb[:, 0:H], op=MUL)
                    nc.vector.tensor_tensor_scan(
                        e_hh[:, s0:s0 + H], e_dA[:, s0:s0 + H],
                        e_dBx[:, s0:s0 + H], initial=0.0, op0=MUL, op1=ADD)

                for s in range(D_STATE):
                    if s == 0:
                        # finish state 0: h1 half only, chained via scan carry
                        Bb, Cb, dA, dBx, hh = e_Bb, e_Cb, e_dA, e_dBx, e_hh
                        nc.sync.dma_start(
                            Bb[:, H:], cc_out[1][64:65, :].to_broadcast([128, H]))
                        nc.sync.dma_start(
                            Cb[:, H:], cc_out[1][80:81, :].to_broadcast([128, H]))
                        for m in range(NT):
                            c0 = SEG[m] * T_LEN
                            nc.scalar.activation(
                                dA[:, c0 + H:c0 + T_LEN], dt_sb[m][:, H:],
                                AF.Exp, scale=a_sb[:, m * 16: m * 16 + 1])
                            nc.vector.tensor_tensor(
                                dBx[:, c0 + H:c0 + T_LEN],
                                u_all[:, c0 + H:c0 + T_LEN],
                                Bb[:, H:], op=MUL)
                            nc.vector.tensor_tensor_scan(
                                hh[:, c0 + H:c0 + T_LEN],
                                dA[:, c0 + H:c0 + T_LEN],
                                dBx[:, c0 + H:c0 + T_LEN],
                                initial=hh[:, c0 + H - 1:c0 + H],
                                op0=MUL, op1=ADD)
                    else:
                        Bb = bcast.tile([128, T_LEN], F16, tag="Bb")
                        Cb = bcast.tile([128, T_LEN], F16, tag="Cb")
                        for h in range(2):
                            nc.sync.dma_start(
                                Bb[:, h * H:(h + 1) * H],
                                cc_out[h][64 + s:65 + s, :].to_broadcast([128, H]))
                            nc.sync.dma_start(
                                Cb[:, h * H:(h + 1) * H],
                                cc_out[h][80 + s:81 + s, :].to_broadcast([128, H]))
                        Bb4 = Bb[:].unsqueeze(1).to_broadcast([128, NT, T_LEN])

                        dA = dap.tile([128, NT * T_LEN], F16, tag="dA")
                        for m in range(NT):
                            nc.scalar.activation(
                                dA[:, seg_sl(m)], dt_sb[m][:], AF.Exp,
                                scale=a_sb[:, m * 16 + s: m * 16 + s + 1])
                        dBx = dbxp.tile([128, NT * T_LEN], F16, tag="dBx")
                        nc.vector.tensor_tensor(
                            dBx[:].rearrange("p (a b) -> p a b", a=NT),
                            u_all[:].rearrange("p (a b) -> p a b", a=NT),
                            Bb4, op=MUL)
                        hh = hp.tile([128, NT * T_LEN], F16, tag="h")
                        for m in range(NT):
                            nc.vector.tensor_tensor_scan(
                                hh[:, seg_sl(m)], dA[:, seg_sl(m)],
                                dBx[:, seg_sl(m)], initial=0.0, op0=MUL, op1=ADD)
                    last = s == D_STATE - 1
                    if s == 0:
                        # hc in place, then seed y (no memset / add)
                        nc.vector.tensor_tensor(
                            hh[:].rearrange("p (a b) -> p a b", a=NT),
                            hh[:].rearrange("p (a b) -> p a b", a=NT),
                            Cb[:].unsqueeze(1).to_broadcast([128, NT, T_LEN]),
                            op=MUL)
                        # seed y_all (global cols 1024..6144)
                        nc.vector.tensor_copy(
                            y_all[:, 0:H], hh[:, H:T_LEN])
                        nc.vector.tensor_copy(
                            y_all[:, H:5 * H], hh[:, T_LEN:3 * T_LEN])
                        for n in range(4):
                            nc.tensor.matmul(
                                y0_ps[:, n * CH:(n + 1) * CH], ident_sb[:],
                                hh[:, 3 * T_LEN + n * CH: 3 * T_LEN + (n + 1) * CH],
                                start=True, stop=False)
                        for n in range(2):
                            nc.tensor.matmul(
                                y1a_ps[:, n * CH:(n + 1) * CH], ident_sb[:],
                                hh[:, n * CH:(n + 1) * CH],
                                start=True, stop=False)
                    elif not last:
                        # hc in place on h, then one wide accumulate
                        Cb4 = Cb[:].unsqueeze(1).to_broadcast([128, NT, T_LEN])
                        nc.vector.tensor_tensor(
                            hh[:].rearrange("p (a b) -> p a b", a=NT),
                            hh[:].rearrange("p (a b) -> p a b", a=NT),
                            Cb4, op=MUL)
                        nc.vector.tensor_tensor(
                            y_all[:], y_all[:], hh[:, H:6 * H], op=ADD)
                        for n in range(4):
                            nc.tensor.matmul(
                                y0_ps[:, n * CH:(n + 1) * CH], ident_sb[:],
                                hh[:, 3 * T_LEN + n * CH: 3 * T_LEN + (n + 1) * CH],
                                start=False, stop=False)
                        for n in range(2):
                            nc.tensor.matmul(
                                y1a_ps[:, n * CH:(n + 1) * CH], ident_sb[:],
                                hh[:, n * CH:(n + 1) * CH],
                                start=False, stop=False)
                    else:
                        # last state: finish per column-quarter and flow each
                        # quarter straight into gate + out_proj so the tail
                        # overlaps the remaining scan work
                        for n in range(4):
                            c0 = n * CH
                            cols = slice(c0, c0 + CH)
                            for m in range(NT):
                                sl = slice(SEG[m] * T_LEN + c0,
                                           SEG[m] * T_LEN + c0 + CH)
                                nc.vector.tensor_tensor(
                                    hh[:, sl], hh[:, sl],
                                    Cb[:, cols], op=MUL)
                                if m != 0 and not (m == 1 and n < 2):
                                    ya = SEG[m] * T_LEN - H + c0
                                    nc.vector.tensor_tensor(
                                        y_all[:, ya:ya + CH],
                                        y_all[:, ya:ya + CH],
                                        hh[:, sl], op=ADD)
                            nc.tensor.matmul(
                                y0_ps[:, cols], ident_sb[:],
                                hh[:, 3 * T_LEN + c0: 3 * T_LEN + c0 + CH],
                                start=False, stop=True)
                            nc.any.tensor_copy(y0[:, cols], y0_ps[:, cols])
                            if n < 2:
                                nc.tensor.matmul(
                                    y1a_ps[:, cols], ident_sb[:],
                                    hh[:, c0:c0 + CH],
                                    start=False, stop=True)
                                nc.any.tensor_copy(y1a[:, cols], y1a_ps[:, cols])
                            # gate for this quarter: xc holds D*conv_x
                            for m in range(NT):
                                if m == 0:
                                    ysl = y0[:, cols]
                                elif m == 1 and n < 2:
                                    ysl = y1a[:, cols]
                                else:
                                    ya = SEG[m] * T_LEN - H + c0
                                    ysl = y_all[:, ya:ya + CH]
                                nc.vector.tensor_tensor(
                                    xc[m][:, cols], xc[m][:, cols],
                                    ysl, op=ADD)
                                nc.vector.tensor_tensor(
                                    xc[m][:, cols], xc[m][:, cols],
                                    zg_all[m][:, cols], op=MUL)
                            # out_proj for this quarter
                            for mo in range(8):
                                pso = psum_mm.tile([128, CH], F32, tag="ps")
                                for k in range(NT):
                                    nc.tensor.matmul(
                                        pso[:],
                                        w_out_sb[:, k * D_MODEL + mo * 128:
                                                 k * D_MODEL + (mo + 1) * 128],
                                        xc[k][:, cols],
                                        start=(k == 0), stop=(k == NT - 1))
                                ot = outstg.tile([128, CH], F32, tag="ot")
                                nc.any.tensor_copy(ot[:], pso[:])
                                nc.sync.dma_start(
                                    out_d[mo * 128:(mo + 1) * 128, cols],
                                    ot[:])

    nc.finalize()
    return nc


def make_in_maps(hidden_states, W_in, conv_w, conv_b, W_x, W_dt, b_dt, A_log, D, W_out):
    f16 = np.float16
    in_maps = []
    for core in range(8):
        g, r = divmod(core, TP)
        sh = slice(r * DSH, (r + 1) * DSH)
        m = {
            "hsT": np.ascontiguousarray(hidden_states[g].T).astype(f16),
            "w_inT": np.ascontiguousarray(
                np.concatenate([W_in[sh], W_in[D_INNER + r * DSH: D_INNER + (r + 1) * DSH]], 0).T).astype(f16),
            "w_xT": np.ascontiguousarray(W_x[:, sh].T).astype(f16),
            "w_dtT": np.ascontiguousarray(W_dt[sh].T).astype(f16),
            "w_outT": np.ascontiguousarray(W_out[:, sh].T).astype(f16),
            "conv_w": np.ascontiguousarray(conv_w[sh, 0, :]).reshape(NT, 128, D_CONV).astype(np.float32),
            "vecs": np.stack([conv_b[sh], b_dt[sh], D[sh], np.zeros_like(conv_b[sh])], -1).reshape(NT, 128, 4).astype(np.float32),
            "a_log": np.ascontiguousarray(A_log[sh]).reshape(NT, 128, D_STATE).astype(np.float32),
            "ident": np.eye(128, dtype=f16),
        }
        in_maps.append(m)
    return in_maps


_NC_CACHE = {}


def kernel(**inputs):
    inputs = {k: np.asarray(v) for k, v in inputs.items()}
    if "nc" not in _NC_CACHE:
        _NC_CACHE["nc"] = build_graph()
    nc = _NC_CACHE["nc"]
    in_maps = make_in_maps(**inputs)
    res = run_bass_kernel_spmd(nc, in_maps, core_ids=list(range(8)))
    outs = res.results
    full = np.zeros((B_SZ, T_LEN, D_MODEL), np.float32)
    for g in range(B_SZ):
        acc = np.zeros((D_MODEL, T_LEN), np.float32)
        for r in range(TP):
            acc += np.asarray(outs[g * TP + r]["out"], np.float32)
        full[g] = acc.T
    return full


# revision 11
# speedup vs baseline: 1.0929x; 1.0028x over previous
"""Mamba block on 8 trn2 NeuronCores — v4.

Sharding: data-parallel over batch (2 groups of 4 cores) x tensor-parallel
over d_inner (4-way, 512 channels/core), [channel, time] layout so the
selective scan runs as `tensor_tensor_scan` along the free (time) axis.

v4 vs v3 (all driven by the DVE being the bottleneck at 77% occupancy):
- B/C per-state broadcasts now run on the DMA engines (stride-0
  partition-broadcast reads straight from the AllReduce output in DRAM)
  instead of PE one-hot matmuls + ScalarE PSUM->SBUF copies. This frees
  2 PSUM banks and ~5us/state of ScalarE.
- Depthwise conv runs on the PE as 4 accumulating diag-matrix matmuls
  (diag built once from `ident` scaled per-partition by the conv tap),
  replacing 24 DVE scalar_tensor_tensor ops. SiLU is applied directly
  PSUM->SBUF on ScalarE via the dedicated Silu activation table entry
  (replaces the 3-op Exp/Ln/Exp chains).
- The gate silu(z) is applied by ScalarE directly on the in_proj PSUM
  output (zg = Silu(z)), removing the DVE z-cast, the deferred 3-op
  sigmoid chain, and the z*sg DVE multiply.
- y accumulation: segment m0 (full T) and the h0 half of segment m1
  accumulate in PSUM via TensorE identity matmuls using the banks freed
  by the DMA broadcasts; the DVE y+= covers the remaining contiguous
  [128, 5*1024] range.
- D skip-connection folded into xc in place with a 4x-mode
  tensor_scalar_mul after u is computed, so the tail per quarter is two
  2x-mode tensor_tensor ops (add y, mul zg).
"""

import numpy as np

from concourse import bass, mybir, tile
from concourse import bacc
from concourse.bass_utils import run_bass_kernel_spmd

D_MODEL = 1024
D_STATE = 16
D_CONV = 4
D_INNER = 2048
DT_RANK = 64
B_SZ, T_LEN = 2, 2048

TP = 4
DSH = D_INNER // TP         # 512 channels per core
NT = DSH // 128             # 4 d-tiles
H = T_LEN // 2              # 1024
CH = 512

F32 = mybir.dt.float32
F16 = mybir.dt.float16
MUL = mybir.AluOpType.mult
ADD = mybir.AluOpType.add
AF = mybir.ActivationFunctionType

SEG = [3, 0, 1, 2]  # m -> segment index in consolidated tiles ([m1,m2,m3,m0])


def build_graph():
    nc = bacc.Bacc("TRN2", target_bir_lowering=False, num_devices=8)

    hsT = nc.dram_tensor("hsT", [D_MODEL, T_LEN], F16, kind="ExternalInput")
    w_inT = nc.dram_tensor("w_inT", [D_MODEL, 2 * DSH], F16, kind="ExternalInput")
    w_xT = nc.dram_tensor("w_xT", [DSH, DT_RANK + 2 * D_STATE], F16, kind="ExternalInput")
    w_dtT = nc.dram_tensor("w_dtT", [DT_RANK, DSH], F16, kind="ExternalInput")
    w_outT = nc.dram_tensor("w_outT", [DSH, D_MODEL], F16, kind="ExternalInput")
    conv_w = nc.dram_tensor("conv_w", [NT, 128, D_CONV], F32, kind="ExternalInput")
    # vecs columns: 0=conv_b, 1=b_dt, 2=D, 3=unused
    vecs = nc.dram_tensor("vecs", [NT, 128, 4], F32, kind="ExternalInput")
    a_log = nc.dram_tensor("a_log", [NT, 128, D_STATE], F32, kind="ExternalInput")
    ident = nc.dram_tensor("ident", [128, 128], F16, kind="ExternalInput")
    out_d = nc.dram_tensor("out", [D_MODEL, T_LEN], F32, kind="ExternalOutput")

    def seg_sl(m):
        s0 = SEG[m] * T_LEN
        return slice(s0, s0 + T_LEN)

    with tile.TileContext(nc) as tc:
        with (
            tc.tile_pool(name="wconst", bufs=1) as wconst,
            tc.tile_pool(name="acts", bufs=1) as acts,
            tc.tile_pool(name="outstg", bufs=2) as outstg,
            tc.tile_pool(name="psum_mm", bufs=2, space="PSUM") as psum_mm,
            tc.tile_pool(name="dram", bufs=1, space="DRAM") as dram,
        ):
            # ---- small resident weights/consts ----
            w_x_sb = wconst.tile([128, NT * 96], F16)
            for k in range(NT):
                nc.sync.dma_start(
                    w_x_sb[:, k * 96:(k + 1) * 96],
                    w_xT[k * 128:(k + 1) * 128, :])
            w_dt_sb = wconst.tile([DT_RANK, DSH], F16)
            nc.sync.dma_start(w_dt_sb[:], w_dtT[:])
            w_out_sb = wconst.tile([128, NT * D_MODEL], F16)
            for k in range(NT):
                nc.sync.dma_start(
                    w_out_sb[:, k * D_MODEL:(k + 1) * D_MODEL],
                    w_outT[k * 128:(k + 1) * 128, :])
            conv_w_sb = wconst.tile([128, NT * 4], F32)
            vecs_sb = wconst.tile([128, NT * 4], F32)
            a_log_sb = wconst.tile([128, NT * D_STATE], F32)
            for m in range(NT):
                nc.sync.dma_start(conv_w_sb[:, m * 4:(m + 1) * 4], conv_w[m])
                nc.sync.dma_start(vecs_sb[:, m * 4:(m + 1) * 4], vecs[m])
                nc.sync.dma_start(a_log_sb[:, m * 16:(m + 1) * 16], a_log[m])
            ident_sb = wconst.tile([128, 128], F16)
            nc.sync.dma_start(ident_sb[:], ident[:])

            a_sb = wconst.tile([128, NT * D_STATE], F32)
            nc.scalar.activation(a_sb[:], a_log_sb[:], AF.Exp)
            nc.vector.tensor_scalar_mul(a_sb[:], a_sb[:], -1.0)

            # conv tap diag matrices: diag(conv_w[:, j]) per d-tile
            diags = wconst.tile([128, NT * 4 * 128], F16)
            for m in range(NT):
                for j in range(4):
                    nc.scalar.activation(
                        diags[:, (m * 4 + j) * 128:(m * 4 + j + 1) * 128],
                        ident_sb[:], AF.Copy,
                        scale=conv_w_sb[:, m * 4 + j: m * 4 + j + 1])

            # ~5us of dummy matmuls releases the PE HAM clock gate before
            # in_proj arrives (cold PE runs at 1.2 GHz vs 2.4 warm)
            with tc.tile_pool(name="warm", bufs=1, space="PSUM") as warmp:
                wps = warmp.tile([128, CH], F32, name="wps")
                for w in range(12):
                    nc.tensor.matmul(
                        wps[:], ident_sb[:], w_out_sb[:, 0:CH],
                        start=True, stop=True)

            # ---- activations ----
            zg_all = [acts.tile([128, T_LEN], F16, name=f"zg{m}") for m in range(NT)]
            xc = [acts.tile([128, T_LEN], F16, name=f"xc{m}") for m in range(NT)]
            dt_sb = [acts.tile([128, T_LEN], F16, name=f"dt{m}") for m in range(NT)]
            u_all = acts.tile([128, NT * T_LEN], F16)
            # y for segments m1(2nd half),m2,m3: global cols 1024..6144
            y_all = acts.tile([128, 5 * H], F16)
            y0 = acts.tile([128, T_LEN], F16)          # m0 (PSUM path)
            y1a = acts.tile([128, H], F16)             # m1 h0-half (PSUM path)

            cc_in = [dram.tile([96, H], F16, name=f"ccin{h}") for h in range(2)]
            cc_out = [dram.tile([96, H], F16, name=f"ccout{h}") for h in range(2)]

            with (
                tc.tile_pool(name="xpool", bufs=1) as xpool,
                tc.tile_pool(name="winp", bufs=1) as winp,
                tc.tile_pool(name="hspool", bufs=2) as hspool,
                tc.tile_pool(name="psum_cv", bufs=2, space="PSUM") as psum_cv,
                tc.tile_pool(name="xdblp", bufs=1) as xdblp,
                tc.tile_pool(name="sppool", bufs=2) as sppool,
            ):
                x_all = [xpool.tile([128, T_LEN + 3], F16, name=f"x{m}")
                         for m in range(NT)]
                w_in_sb = winp.tile([128, 8 * 1024], F16)
                for k in range(8):
                    nc.sync.dma_start(
                        w_in_sb[:, k * 1024:(k + 1) * 1024],
                        w_inT[k * 128:(k + 1) * 128, :])
                for m in range(NT):
                    nc.any.memset(x_all[m][:, 0:3], 0.0)

                for h in range(2):
                    hc0 = h * H
                    # ---- P1: in_proj ----
                    for n in range(2):
                        cols = slice(hc0 + n * CH, hc0 + (n + 1) * CH)
                        hs_n = hspool.tile([128, 8 * CH], F16, tag="hs")
                        for k in range(8):
                            nc.sync.dma_start(
                                hs_n[:, k * CH:(k + 1) * CH],
                                hsT[k * 128:(k + 1) * 128, cols])
                        # mo in pairs: alternate two PSUM tiles so back-to-back
                        # matmuls never hit the same accumulation chain and the
                        # PE SBUF access latency stays hidden
                        for g in range(4):
                            psa = psum_mm.tile([128, CH], F32, tag="ps")
                            psb = psum_mm.tile([128, CH], F32, tag="ps")
                            for k in range(8):
                                for i, ps in ((0, psa), (1, psb)):
                                    mo = 2 * g + i
                                    nc.tensor.matmul(
                                        ps[:],
                                        w_in_sb[:, k * 1024 + mo * 128:
                                                k * 1024 + (mo + 1) * 128],
                                        hs_n[:, k * CH:(k + 1) * CH],
                                        start=(k == 0), stop=(k == 7))
                            for i, ps in ((0, psa), (1, psb)):
                                mo = 2 * g + i
                                if mo < NT:
                                    nc.vector.tensor_copy(
                                        x_all[mo][:, 3 + hc0 + n * CH:
                                                  3 + hc0 + (n + 1) * CH], ps[:])
                                else:
                                    # gate: zg = silu(z) straight from PSUM
                                    nc.scalar.activation(
                                        zg_all[mo - NT][:, cols], ps[:], AF.Silu)

                    # ---- P2: conv on PE + silu(x) from PSUM ----
                    for m in range(NT):
                        cps = psum_cv.tile([128, H], F32, tag="cv")
                        for j in range(D_CONV):
                            for c in range(2):
                                nc.tensor.matmul(
                                    cps[:, c * CH:(c + 1) * CH],
                                    diags[:, (m * 4 + j) * 128:
                                          (m * 4 + j + 1) * 128],
                                    x_all[m][:, hc0 + j + c * CH:
                                             hc0 + j + c * CH + CH],
                                    start=(j == 0), stop=(j == D_CONV - 1))
                        nc.scalar.activation(
                            xc[m][:, hc0:hc0 + H], cps[:], AF.Silu,
                            bias=vecs_sb[:, m * 4: m * 4 + 1])

                    # ---- P3: x_proj partial + AllReduce(h) (f16) ----
                    xdbl_h = xdblp.tile([96, H], F16, tag="xd")
                    psxs = [psum_mm.tile([128, CH], F32, tag="ps",
                                         name=f"psx{n}")
                            for n in range(2)]
                    for k in range(NT):
                        for n in range(2):
                            cols = slice(hc0 + n * CH, hc0 + (n + 1) * CH)
                            nc.tensor.matmul(
                                psxs[n][0:96, :],
                                w_x_sb[:, k * 96:(k + 1) * 96],
                                xc[k][:, cols],
                                start=(k == 0), stop=(k == NT - 1))
                    for n in range(2):
                        nc.vector.tensor_copy(
                            xdbl_h[:, n * CH:(n + 1) * CH], psxs[n][0:96, :])
                    nc.sync.dma_start(cc_in[h][:], xdbl_h[:])
                    nc.gpsimd.collective_compute(
                        "AllReduce", ADD,
                        replica_groups=[[0, 1, 2, 3], [4, 5, 6, 7]],
                        ins=[cc_in[h].opt()], outs=[cc_out[h].opt()])

                # ---- P4 per half: AR readback, dt_proj+softplus, u, D-fold ----
                for h in range(2):
                    hc0 = h * H
                    xdbl16 = xdblp.tile([DT_RANK, H], F16, tag="x16")
                    nc.sync.dma_start(xdbl16[:], cc_out[h][0:64, :])
                    # all Exp ops first, then all Ln ops — interleaving them
                    # makes the act-table pass thrash 1.3us table loads
                    sp_h = sppool.tile([128, 8 * CH], F16, tag="sp")
                    for m in range(NT):
                        for n in range(2):
                            psd = psum_mm.tile([128, CH], F32, tag="ps")
                            nc.tensor.matmul(
                                psd[:], w_dt_sb[:, m * 128:(m + 1) * 128],
                                xdbl16[:, n * CH:(n + 1) * CH],
                                start=True, stop=True)
                            nc.scalar.activation(
                                sp_h[:, (2 * m + n) * CH:(2 * m + n + 1) * CH],
                                psd[:], AF.Exp,
                                bias=vecs_sb[:, m * 4 + 1: m * 4 + 2])
                    for m in range(NT):
                        for n in range(2):
                            cols = slice(hc0 + n * CH, hc0 + (n + 1) * CH)
                            nc.scalar.activation(
                                dt_sb[m][:, cols],
                                sp_h[:, (2 * m + n) * CH:(2 * m + n + 1) * CH],
                                AF.Ln, bias=1.0)
                    for m in range(NT):
                        nc.vector.tensor_tensor(
                            u_all[:, SEG[m] * T_LEN + hc0:
                                  SEG[m] * T_LEN + hc0 + H],
                            dt_sb[m][:, hc0:hc0 + H],
                            xc[m][:, hc0:hc0 + H], op=MUL)
                        # fold D into xc in place (xc no longer needed raw)
                        nc.vector.tensor_scalar_mul(
                            xc[m][:, hc0:hc0 + H], xc[m][:, hc0:hc0 + H],
                            vecs_sb[:, m * 4 + 2: m * 4 + 3])

            # ---- P6: selective scan over states (full T) ----
            with (
                tc.tile_pool(name="bcast", bufs=2) as bcast,
                tc.tile_pool(name="dap", bufs=2) as dap,
                tc.tile_pool(name="dbxp", bufs=1) as dbxp,
                tc.tile_pool(name="hp", bufs=2) as hp,
                tc.tile_pool(name="psum_y", bufs=1, space="PSUM") as psum_y,
            ):
                y0_ps = psum_y.tile([128, T_LEN], F32, name="y0_ps")
                y1a_ps = psum_y.tile([128, H], F32, name="y1a_ps")

                # Early block: state 0's h0 half only needs AR(h0)-derived
                # data, ready while AR(h1) is still in flight.
                e_Bb = bcast.tile([128, T_LEN], F16, tag="Bb")
                e_Cb = bcast.tile([128, T_LEN], F16, tag="Cb")
                nc.sync.dma_start(
                    e_Bb[:, 0:H], cc_out[0][64:65, :].to_broadcast([128, H]))
                nc.sync.dma_start(
                    e_Cb[:, 0:H], cc_out[0][80:81, :].to_broadcast([128, H]))
                e_dA = dap.tile([128, NT * T_LEN], F16, tag="dA")
                e_dBx = dbxp.tile([128, NT * T_LEN], F16, tag="dBx")
                e_hh = hp.tile([128, NT * T_LEN], F16, tag="h")
                for m in range(NT):
                    s0 = SEG[m] * T_LEN
                    nc.scalar.activation(
                        e_dA[:, s0:s0 + H], dt_sb[m][:, 0:H], AF.Exp,
                        scale=a_sb[:, m * 16: m * 16 + 1])
                    nc.vector.tensor_tensor(
                        e_dBx[:, s0:s0 + H], u_all[:, s0:s0 + H],
                        e_Bb[:, 0:H], op=MUL)
                    nc.vector.tensor_tensor_scan(
                        e_hh[:, s0:s0 + H], e_dA[:, s0:s0 + H],
                        e_dBx[:, s0:s0 + H], initial=0.0, op0=MUL, op1=ADD)

                for s in range(D_STATE):
                    if s == 0:
                        # finish state 0: h1 half only, chained via scan carry
                        Bb, Cb, dA, dBx, hh = e_Bb, e_Cb, e_dA, e_dBx, e_hh
                        nc.sync.dma_start(
                            Bb[:, H:], cc_out[1][64:65, :].to_broadcast([128, H]))
                        nc.sync.dma_start(
                            Cb[:, H:], cc_out[1][80:81, :].to_broadcast([128, H]))
                        for m in range(NT):
                            c0 = SEG[m] * T_LEN
                            nc.scalar.activation(
                                dA[:, c0 + H:c0 + T_LEN], dt_sb[m][:, H:],
                                AF.Exp, scale=a_sb[:, m * 16: m * 16 + 1])
                            nc.vector.tensor_tensor(
                                dBx[:, c0 + H:c0 + T_LEN],
                                u_all[:, c0 + H:c0 + T_LEN],
                                Bb[:, H:], op=MUL)
                            nc.vector.tensor_tensor_scan(
                                hh[:, c0 + H:c0 + T_LEN],
                                dA[:, c0 + H:c0 + T_LEN],
                                dBx[:, c0 + H:c0 + T_LEN],
                                initial=hh[:, c0 + H - 1:c0 + H],
                                op0=MUL, op1=ADD)
                    else:
                        Bb = bcast.tile([128, T_LEN], F16, tag="Bb")
                        Cb = bcast.tile([128, T_LEN], F16, tag="Cb")
                        for h in range(2):
                            nc.sync.dma_start(
                                Bb[:, h * H:(h + 1) * H],
                                cc_out[h][64 + s:65 + s, :].to_broadcast([128, H]))
                            nc.sync.dma_start(
                                Cb[:, h * H:(h + 1) * H],
                                cc_out[h][80 + s:81 + s, :].to_broadcast([128, H]))
                        Bb4 = Bb[:].unsqueeze(1).to_broadcast([128, NT, T_LEN])

                        dA = dap.tile([128, NT * T_LEN], F16, tag="dA")
                        for m in range(NT):
                            nc.scalar.activation(
                                dA[:, seg_sl(m)], dt_sb[m][:], AF.Exp,
                                scale=a_sb[:, m * 16 + s: m * 16 + s + 1])
                        dBx = dbxp.tile([128, NT * T_LEN], F16, tag="dBx")
                        nc.vector.tensor_tensor(
                            dBx[:].rearrange("p (a b) -> p a b", a=NT),
                            u_all[:].rearrange("p (a b) -> p a b", a=NT),
                            Bb4, op=MUL)
                        hh = hp.tile([128, NT * T_LEN], F16, tag="h")
                        for m in range(NT):
                            nc.vector.tensor_tensor_scan(
                                hh[:, seg_sl(m)], dA[:, seg_sl(m)],
                                dBx[:, seg_sl(m)], initial=0.0, op0=MUL, op1=ADD)
                    last = s == D_STATE - 1
                    if s == 0:
                        # hc in place, then seed y (no memset / add)
                        nc.vector.tensor_tensor(
                            hh[:].rearrange("p (a b) -> p a b", a=NT),
                            hh[:].rearrange("p (a b) -> p a b", a=NT),
                            Cb[:].unsqueeze(1).to_broadcast([128, NT, T_LEN]),
                            op=MUL)
                        # seed y_all (global cols 1024..6144)
                        nc.vector.tensor_copy(
                            y_all[:, 0:H], hh[:, H:T_LEN])
                        nc.vector.tensor_copy(
                            y_all[:, H:5 * H], hh[:, T_LEN:3 * T_LEN])
                        for n in range(4):
                            nc.tensor.matmul(
                                y0_ps[:, n * CH:(n + 1) * CH], ident_sb[:],
                                hh[:, 3 * T_LEN + n * CH: 3 * T_LEN + (n + 1) * CH],
                                start=True, stop=False)
                        for n in range(2):
                            nc.tensor.matmul(
                                y1a_ps[:, n * CH:(n + 1) * CH], ident_sb[:],
                                hh[:, n * CH:(n + 1) * CH],
                                start=True, stop=False)
                    elif not last:
                        # hc in place on h, then one wide accumulate
                        Cb4 = Cb[:].unsqueeze(1).to_broadcast([128, NT, T_LEN])
                        nc.vector.tensor_tensor(
                            hh[:].rearrange("p (a b) -> p a b", a=NT),
                            hh[:].rearrange("p (a b) -> p a b", a=NT),
                            Cb4, op=MUL)
                        nc.vector.tensor_tensor(
                            y_all[:], y_all[:], hh[:, H:6 * H], op=ADD)
                        for n in range(4):
                            nc.tensor.matmul(
                                y0_ps[:, n * CH:(n + 1) * CH], ident_sb[:],
                                hh[:, 3 * T_LEN + n * CH: 3 * T_LEN + (n + 1) * CH],
                                start=False, stop=False)
                        for n in range(2):
                            nc.tensor.matmul(
                                y1a_ps[:, n * CH:(n + 1) * CH], ident_sb[:],
                                hh[:, n * CH:(n + 1) * CH],
                                start=False, stop=False)
                    else:
                        # last state: finish per column-quarter and flow each
                        # quarter straight into gate + out_proj so the tail
                        # overlaps the remaining scan work
                        for n in range(4):
                            c0 = n * CH
                            cols = slice(c0, c0 + CH)
                            for m in range(NT):
                                sl = slice(SEG[m] * T_LEN + c0,
                                           SEG[m] * T_LEN + c0 + CH)
                                nc.vector.tensor_tensor(
                                    hh[:, sl], hh[:, sl],
                                    Cb[:, cols], op=MUL)
                                if m != 0 and not (m == 1 and n < 2):
                                    ya = SEG[m] * T_LEN - H + c0
                                    nc.vector.tensor_tensor(
                                        y_all[:, ya:ya + CH],
                                        y_all[:, ya:ya + CH],
                                        hh[:, sl], op=ADD)
                            nc.tensor.matmul(
                                y0_ps[:, cols], ident_sb[:],
                                hh[:, 3 * T_LEN + c0: 3 * T_LEN + c0 + CH],
                                start=False, stop=True)
                            nc.any.tensor_copy(y0[:, cols], y0_ps[:, cols])
                            if n < 2:
                                nc.tensor.matmul(
                                    y1a_ps[:, cols], ident_sb[:],
                                    hh[:, c0:c0 + CH],
                                    start=False, stop=True)
                                nc.any.tensor_copy(y1a[:, cols], y1a_ps[:, cols])
                            # gate for this quarter: xc holds D*conv_x
                            for m in range(NT):
                                if m == 0:
                                    ysl = y0[:, cols]
                                elif m == 1 and n < 2:
                                    ysl = y1a[:, cols]
                                else:
                                    ya = SEG[m] * T_LEN - H + c0
                                    ysl = y_all[:, ya:ya + CH]
                                nc.vector.tensor_tensor(
                                    xc[m][:, cols], xc[m][:, cols],
                                    ysl, op=ADD)
                                nc.vector.tensor_tensor(
                                    xc[m][:, cols], xc[m][:, cols],
                                    zg_all[m][:, cols], op=MUL)
                            # out_proj for this quarter, mo in pairs so
                            # back-to-back matmuls hit different PSUM tiles
                            for g in range(4):
                                psa = psum_mm.tile([128, CH], F32, tag="ps")
                                psb = psum_mm.tile([128, CH], F32, tag="ps")
                                for k in range(NT):
                                    for i, pso in ((0, psa), (1, psb)):
                                        mo = 2 * g + i
                                        nc.tensor.matmul(
                                            pso[:],
                                            w_out_sb[:, k * D_MODEL + mo * 128:
                                                     k * D_MODEL + (mo + 1) * 128],
                                            xc[k][:, cols],
                                            start=(k == 0), stop=(k == NT - 1))
                                for i, pso in ((0, psa), (1, psb)):
                                    mo = 2 * g + i
                                    ot = outstg.tile([128, CH], F32, tag="ot")
                                    nc.any.tensor_copy(ot[:], pso[:])
                                    nc.sync.dma_start(
                                        out_d[mo * 128:(mo + 1) * 128, cols],
                                        ot[:])

    nc.finalize()
    return nc


def make_in_maps(hidden_states, W_in, conv_w, conv_b, W_x, W_dt, b_dt, A_log, D, W_out):
    f16 = np.float16
    in_maps = []
    for core in range(8):
        g, r = divmod(core, TP)
        sh = slice(r * DSH, (r + 1) * DSH)
        m = {
            "hsT": np.ascontiguousarray(hidden_states[g].T).astype(f16),
            "w_inT": np.ascontiguousarray(
                np.concatenate([W_in[sh], W_in[D_INNER + r * DSH: D_INNER + (r + 1) * DSH]], 0).T).astype(f16),
            "w_xT": np.ascontiguousarray(W_x[:, sh].T).astype(f16),
            "w_dtT": np.ascontiguousarray(W_dt[sh].T).astype(f16),
            "w_outT": np.ascontiguousarray(W_out[:, sh].T).astype(f16),
            "conv_w": np.ascontiguousarray(conv_w[sh, 0, :]).reshape(NT, 128, D_CONV).astype(np.float32),
            "vecs": np.stack([conv_b[sh], b_dt[sh], D[sh], np.zeros_like(conv_b[sh])], -1).reshape(NT, 128, 4).astype(np.float32),
            "a_log": np.ascontiguousarray(A_log[sh]).reshape(NT, 128, D_STATE).astype(np.float32),
            "ident": np.eye(128, dtype=f16),
        }
        in_maps.append(m)
    return in_maps


_NC_CACHE = {}


def kernel(**inputs):
    inputs = {k: np.asarray(v) for k, v in inputs.items()}
    if "nc" not in _NC_CACHE:
        _NC_CACHE["nc"] = build_graph()
    nc = _NC_CACHE["nc"]
    in_maps = make_in_maps(**inputs)
    res = run_bass_kernel_spmd(nc, in_maps, core_ids=list(range(8)))
    outs = res.results
    full = np.zeros((B_SZ, T_LEN, D_MODEL), np.float32)
    for g in range(B_SZ):
        acc = np.zeros((D_MODEL, T_LEN), np.float32)
        for r in range(TP):
            acc += np.asarray(outs[g * TP + r]["out"], np.float32)
        full[g] = acc.T
    return full
